# revision 81
# baseline (speedup 1.0000x reference)
"""Trainium2 Bass kernel for nn_Attention (dense transformer self-attention block).

Reference computation (fp32):
    xn = LayerNorm(x) * gamma
    q = (xn @ w_q) * SCALE ; k, v = split(xn @ w_kv, 2)
    k/v get a null key/value prepended; sim = q @ k^T; masked softmax
    out = (softmax(sim) @ v) @ w_out

Sharding: 8 cores = 4 batches x 2 head-groups. Core (b, hg) computes
LayerNorm + q/k/v projections + attention for batch b and heads
hg*8..hg*8+8 (attention is fully local to a batch), then projects all 2048
rows through its 512 rows of w_out; a pairwise ReduceScatter sums the two
partial outputs and leaves each core with its half of the rows.

All matmuls run in bf16 with fp32 PSUM accumulation; LayerNorm statistics and
softmax denominators stay fp32.  Softmax is computed without max-subtraction
(sim ~ N(0,1) here, exp cannot overflow) as exp(sim) divided by the row sum,
which is obtained by appending an all-ones column to v.  The context mask and
the null key are folded into the (padded) key/value tensors, which reproduces
the reference masking exactly.
"""

import numpy as np
import ml_dtypes

B = 4
N = 2048
DIM = 1024
H_ALL = 16
DH = 64
INNER = H_ALL * DH
SCALE = DH ** -0.5
EPS = 1e-5

H = 8            # heads per core
G = 4            # head groups of 2 per core (128 partitions each)
R = N            # rows (sequence) per batch
J = 2048         # key slots (the null key is handled as a rank-1 update)
NT_J = J // 128  # 16
IC = 1024        # attention i-chunk (query columns per pass)
N_CORES = 8

_CACHE = {}


def _build(repeat=1):
    from contextlib import ExitStack, nullcontext
    from concourse import bacc
    import concourse.tile as tile
    from concourse import mybir

    f32 = mybir.dt.float32
    bf16 = mybir.dt.bfloat16
    AF = mybir.ActivationFunctionType
    OP = mybir.AluOpType

    nc = bacc.Bacc()

    x_in = nc.declare_dram_parameter("x", [R, DIM], f32, isOutput=False)
    wq_in = nc.declare_dram_parameter("wq", [DIM, 512], bf16, isOutput=False)
    wk_in = nc.declare_dram_parameter("wk", [DIM, 512], bf16, isOutput=False)
    wv_in = nc.declare_dram_parameter("wv", [DIM, 512], bf16, isOutput=False)
    wo_in = nc.declare_dram_parameter("wo", [512, DIM], bf16, isOutput=False)
    nk_in = nc.declare_dram_parameter("nk", [G, 128, DH + 1], bf16, isOutput=False)
    nv_in = nc.declare_dram_parameter("nv", [1, H, DH + 1], bf16, isOutput=False)
    mask_in = nc.declare_dram_parameter("maskf", [R], f32, isOutput=False)
    out_ext = nc.declare_dram_parameter("out", [N // 2, DIM], mybir.dt.float16, isOutput=True)

    xn_dram = nc.dram_tensor("xn_dram", [R, DIM], bf16)
    recip_dram = nc.dram_tensor("recip_dram", [H * 2, IC], mybir.dt.float32)
    part_dram = nc.dram_tensor("part_dram", [R, DIM], mybir.dt.float16)
    rs_dram = nc.dram_tensor("rs_dram", [N // 2, DIM], mybir.dt.float16)

    with tile.TileContext(nc) as tc, ExitStack() as ctx:
        # ---- persistent SBUF tensors --------------------------------------
        persist = ctx.enter_context(tc.tile_pool(name="persist", bufs=1))
        wq_sb = [persist.tile([128, 512], bf16, tag=f"wq{t}", name=f"wq{t}") for t in range(8)]
        wk_sb = [persist.tile([128, 512], bf16, tag=f"wk{t}", name=f"wk{t}") for t in range(8)]
        wv_sb = [persist.tile([128, 512], bf16, tag=f"wv{t}", name=f"wv{t}") for t in range(8)]
        wo_sb = [persist.tile([128, DIM], bf16, tag=f"wo{t}", name=f"wo{t}") for t in range(G)]
        qT_sb = [persist.tile([128, R], bf16, tag=f"qT{g}", name=f"qT{g}") for g in range(G)]
        kT_sb = [persist.tile([128, J], bf16, tag=f"kT{g}", name=f"kT{g}") for g in range(G)]
        v_sb = [persist.tile([128, H, DH + 1], bf16, tag=f"v{t}", name=f"v{t}") for t in range(16)]
        att_sb = [persist.tile([128, R], bf16, tag=f"att{g}", name=f"att{g}") for g in range(G)]
        mask_sb = persist.tile([128, 16], f32, tag="mask", name="mask")
        nk_sb = persist.tile([128, G, DH + 1], bf16, tag="nk", name="nk")
        nv_sb = persist.tile([128, H, DH + 1], bf16, tag="nv", name="nv")


        nc.sync.dma_start(out=nv_sb[0:1], in_=nv_in[:])
        nc.sync.dma_start(out=nv_sb[64:65], in_=nv_in[:])

        # ---- LayerNorm + write xn (bf16) back to DRAM ---------------------
        eps_t = persist.tile([128, 1], f32, tag="eps", name="eps")
        nc.vector.memset(eps_t, EPS)
        xnT = [persist.tile([128, R], bf16, tag=f"xnT{c}", name=f"xnT{c}")
               for c in range(8)]
        for _rep in range(repeat):
            _phases(nc, tc, tile, mybir, locals())

    nc.finalize()
    return nc


def _phases(nc, tc, tile, mybir, env):
    f32 = mybir.dt.float32
    f16 = mybir.dt.float16
    bf16 = mybir.dt.bfloat16
    AF = mybir.ActivationFunctionType
    OP = mybir.AluOpType
    (x_in, xn_dram, part_dram, rs_dram, out_ext) = (
        env["x_in"], env["xn_dram"], env["part_dram"], env["rs_dram"], env["out_ext"])
    (wq_sb, wk_sb, wv_sb, wo_sb, qT_sb, kT_sb, v_sb, att_sb, mask_sb, nk_sb,
     nv_sb, eps_t, xnT) = (
        env["wq_sb"], env["wk_sb"], env["wv_sb"], env["wo_sb"], env["qT_sb"],
        env["kT_sb"], env["v_sb"], env["att_sb"], env["mask_sb"], env["nk_sb"],
        env["nv_sb"], env["eps_t"], env["xnT"])
    recip_dram = env["recip_dram"]
    (wq_in, wk_in, wv_in, wo_in, mask_in, nk_in) = (
        env["wq_in"], env["wk_in"], env["wv_in"], env["wo_in"],
        env["mask_in"], env["nk_in"])

    # ---- LayerNorm (per 128-row tile) + chunked DMA-xbar transposes -------
    # Transposes are split by 512-row chunks so they start as soon as the
    # first four LN tiles have landed in DRAM, and projections start as soon
    # as the first chunks of xnT exist.
    with tc.tile_pool(name="ln", bufs=6) as ln_pool, \
         tc.tile_pool(name="ln_small", bufs=6) as ln_small:
        for t in range(R // 128):
            x_t = ln_pool.tile([128, DIM], f32, tag="x", bufs=12)
            eng = nc.sync if t % 2 == 0 else nc.scalar
            eng.dma_start(out=x_t, in_=x_in[t * 128:(t + 1) * 128, :])
            stats = ln_small.tile([128, 2, 6], f32, tag="stats")
            for s in range(2):
                nc.vector.bn_stats(out=stats[:, s, :],
                                   in_=x_t[:, s * 512:(s + 1) * 512])
            mv = ln_small.tile([128, 2], f32, tag="mv")
            nc.vector.bn_aggr(out=mv, in_=stats)
            rstd = ln_small.tile([128, 1], f32, tag="rstd")
            nc.scalar.activation(out=rstd, in_=mv[:, 1:2], func=AF.Sqrt,
                                 bias=eps_t, scale=1.0)
            nc.vector.reciprocal(out=rstd, in_=rstd)
            nbias = ln_small.tile([128, 1], f32, tag="nbias")
            nc.vector.tensor_scalar(out=nbias, in0=rstd,
                                    scalar1=mv[:, 0:1], scalar2=-1.0,
                                    op0=OP.mult, op1=OP.mult)
            xn_t = ln_pool.tile([128, DIM], bf16, tag="xn")
            nc.scalar.activation(out=xn_t, in_=x_t, func=AF.Identity,
                                 bias=nbias, scale=rstd)
            if t == 4:
                for wt in range(8):
                    nc.gpsimd.dma_start(out=wk_sb[wt],
                                        in_=wk_in[wt * 128:(wt + 1) * 128, :])
                    nc.gpsimd.dma_start(out=wq_sb[wt],
                                        in_=wq_in[wt * 128:(wt + 1) * 128, :])
                nc.sync.dma_start(out=mask_sb,
                                  in_=mask_in.rearrange("(t p) -> p t", p=128))
                nc.sync.dma_start(out=nk_sb,
                                  in_=nk_in.rearrange("g p c -> p g c"))
                for wt in range(8):
                    nc.gpsimd.dma_start(out=wv_sb[wt],
                                        in_=wv_in[wt * 128:(wt + 1) * 128, :])
            nc.sync.dma_start(out=xn_dram[t * 128:(t + 1) * 128, :], in_=xn_t)
            if t % 2 == 1:
                rc = t // 2
                for c in range(8):
                    nc.sync.dma_start_transpose(
                        out=xnT[c][:, rc * 256:(rc + 1) * 256],
                        in_=xn_dram[rc * 256:(rc + 1) * 256,
                                    c * 128:(c + 1) * 128])

    # ---- projections + attention, interleaved per head-group -------------
    # v first (attention needs all of v), then per group g: k[g], q[g],
    # attention for g.  The scheduler overlaps the next group's projections
    # with the current group's (ACT-bound) attention inner loop.
    with tc.tile_pool(name="ps_proj", bufs=2, space="PSUM") as ps_proj, \
         tc.tile_pool(name="ps_sim", bufs=2, space="PSUM") as ps_sim, \
         tc.tile_pool(name="ps_av", bufs=1, space="PSUM") as ps_av, \
         tc.tile_pool(name="exp_pool", bufs=8) as exp_pool, \
         tc.tile_pool(name="att_small", bufs=4) as att_small:

        def project_v(jt):
            # v: [key rows on partitions, 8 heads x 64 dims free], mask folded
            ps = ps_proj.tile([128, 512], f32, tag="ps", name="ps")
            for kt in range(8):
                nc.tensor.matmul(
                    ps,
                    lhsT=xnT[kt][:, jt * 128:(jt + 1) * 128],
                    rhs=wv_sb[kt],
                    start=(kt == 0), stop=(kt == 7))
            nc.vector.tensor_scalar_mul(
                out=v_sb[jt][:, :, 0:DH],
                in0=ps.rearrange("p (l d) -> p l d", l=H),
                scalar1=mask_sb[:, jt:jt + 1])
            nc.vector.memset(v_sb[jt][:, :, DH:DH + 1], 0.0)
            nc.vector.tensor_scalar_add(
                out=v_sb[jt][:, :, DH:DH + 1],
                in0=v_sb[jt][:, :, DH:DH + 1],
                scalar1=mask_sb[:, jt:jt + 1])

        def project_kq_piece(g, piece):
            # k[g] / q[g] one (dst, rc) chunk: [2-head dims on parts, rows]
            dst_sb, w_sb = ((kT_sb, wk_sb), (qT_sb, wq_sb))[piece // 4]
            rc = piece % 4
            ps = ps_proj.tile([128, 512], f32, tag="ps", name="ps")
            for kt in range(8):
                nc.tensor.matmul(
                    ps,
                    lhsT=w_sb[kt][:, g * 128:(g + 1) * 128],
                    rhs=xnT[kt][:, rc * 512:(rc + 1) * 512],
                    start=(kt == 0), stop=(kt == 7))
            nc.vector.tensor_copy(
                out=dst_sb[g][:, rc * 512:(rc + 1) * 512], in_=ps)


        def P(g, piece):
            return lambda: project_kq_piece(g, piece)

        def V(jt):
            return lambda: project_v(jt)



        null_expn = {}

        def attention(g, icc, w, inlines=()):
            l = 2 * g + w  # local head index
            i0 = icc * IC
            av = ps_av.tile([DH + 1, IC], f32, tag="av", name="av")
            for jt in range(NT_J):
                if 1 <= jt <= len(inlines):
                    inlines[jt - 1]()
                sim = ps_sim.tile([128, IC], f32, tag="sim", name="sim")
                _sv = tc.cur_priority
                tc.cur_priority = max(0, _sv - 250)
                for nn in range(IC // 512):
                    nc.tensor.matmul(
                        sim[:, nn * 512:(nn + 1) * 512],
                        lhsT=kT_sb[g][w * 64:(w + 1) * 64,
                                      jt * 128:(jt + 1) * 128],
                        rhs=qT_sb[g][w * 64:(w + 1) * 64,
                                     i0 + nn * 512:i0 + (nn + 1) * 512],
                        start=True, stop=True,
                        skip_group_check=True)
                tc.cur_priority = _sv
                exp_t = exp_pool.tile([128, IC], bf16, tag="exp", name="exp")
                nc.scalar.activation(out=exp_t, in_=sim, func=AF.Exp)
                for nn in range(IC // 512):
                    nc.tensor.matmul(
                        av[:, nn * 512:(nn + 1) * 512],
                        lhsT=v_sb[jt][:, l, :],
                        rhs=exp_t[:, nn * 512:(nn + 1) * 512],
                        start=(jt == 0), stop=False,
                        skip_group_check=True)
            if w == 0:
                # null-key sims for BOTH heads of the group in one matmul:
                # head w's sim lands on psum partition w*64 (nk65 layout).
                nsim = ps_sim.tile([128, IC], f32, tag="sim", name="nsim")
                _sv2 = tc.cur_priority
                tc.cur_priority = max(0, _sv2 - 250)
                for nn in range(IC // 512):
                    nc.tensor.matmul(
                        nsim[0:DH + 1, nn * 512:(nn + 1) * 512],
                        lhsT=nk_sb[:, g, :],
                        rhs=qT_sb[g][:, i0 + nn * 512:i0 + (nn + 1) * 512],
                        start=True, stop=True, skip_group_check=True)
                tc.cur_priority = _sv2
                expn = exp_pool.tile([128, IC], bf16, tag="exp", name="expn")
                nc.scalar.activation(out=expn[0:DH + 1, :],
                                     in_=nsim[0:DH + 1, :], func=AF.Exp)
                null_expn[(g, icc)] = expn
            expn = null_expn[(g, icc)]
            for nn in range(IC // 512):
                nc.tensor.matmul(
                    av[:, nn * 512:(nn + 1) * 512],
                    lhsT=nv_sb[w * 64:w * 64 + 1, l, :],
                    rhs=expn[w * 64:w * 64 + 1, nn * 512:(nn + 1) * 512],
                    start=False, stop=True, skip_group_check=True)
            # evict unnormalized (frees the psum accumulator fast), save the
            # denominator; the reciprocal/broadcast/multiply runs off the
            # critical path on DVE + DMA.
            nc.vector.tensor_copy(
                out=att_sb[g][w * 64:(w + 1) * 64, i0:i0 + IC],
                in_=av[0:DH, :])
            recip = att_small.tile([1, IC], f32, tag="recip", name="recip")
            nc.vector.reciprocal(out=recip, in_=av[DH:DH + 1, :])
            slot = recip_dram[l * 2 + icc:l * 2 + icc + 1, :]
            nc.sync.dma_start(out=slot, in_=recip)
            bcast = att_small.tile([128, IC], f32, tag="bcast", name="bcast")
            bslice = bcast[w * 64:(w + 1) * 64, :]
            nc.sync.dma_start(out=bslice, in_=slot.to_broadcast([64, IC]))
            nc.vector.tensor_mul(
                out=att_sb[g][w * 64:(w + 1) * 64, i0:i0 + IC],
                in0=att_sb[g][w * 64:(w + 1) * 64, i0:i0 + IC], in1=bslice)

        def outproj_m(m):
            for oc in range(2):
                ps = ps_proj.tile([128, 512], f32, tag="ps", name="pso")
                for g in range(G):
                    nc.tensor.matmul(
                        ps,
                        lhsT=att_sb[g][:, m * 128:(m + 1) * 128],
                        rhs=wo_sb[g][:, oc * 512:(oc + 1) * 512],
                        start=(g == 0), stop=(g == G - 1))
                o_t = op_out.tile([128, 512], f16, tag="ot", name="ot")
                nc.vector.tensor_copy(out=o_t, in_=ps)
                nc.sync.dma_start(
                    out=part_dram[m * 128:(m + 1) * 128,
                                  oc * 512:(oc + 1) * 512],
                    in_=o_t)

        def rs_quarter(qr):
            nc.gpsimd.collective_compute(
                "ReduceScatter", OP.add,
                replica_groups=[[0, 1], [2, 3], [4, 5], [6, 7]],
                ins=[part_dram[qr * 512:(qr + 1) * 512, :]],
                outs=[rs_dram[qr * 256:(qr + 1) * 256, :]])
            for q in range(2):
                r0 = qr * 256 + q * 128
                ob = op_out.tile([128, DIM], f16, tag="ob", name="ob")
                nc.gpsimd.dma_start(out=ob, in_=rs_dram[r0:r0 + 128, :])
                nc.gpsimd.dma_start(out=out_ext[r0:r0 + 128, :], in_=ob)

        with tc.tile_pool(name="op_out", bufs=6) as op_out:
            # icc0 sweep with just-in-time k/q projections; v split around the
            # first attention block so the exp stream starts early and never
            # starves while v finishes.
            for pc in (0, 4, 5):   # k0 rc0, q0 rc0, q0 rc1
                project_kq_piece(0, pc)
            for jt in range(8):
                project_v(jt)
            attention(0, 0, 0, inlines=(
                P(0, 1), P(0, 2), P(0, 3),
                V(8), V(9), V(10), V(11), V(12), V(13), V(14), V(15)))
            attention(0, 0, 1, inlines=(P(1, 0), P(1, 4), P(1, 5)))
            attention(1, 0, 0, inlines=(P(1, 1), P(1, 2), P(1, 3)))
            attention(1, 0, 1, inlines=(P(2, 0), P(2, 4), P(2, 5)))
            attention(2, 0, 0, inlines=(P(2, 1), P(2, 2), P(2, 3)))
            attention(2, 0, 1, inlines=(P(3, 0), P(3, 4), P(3, 5)))
            attention(3, 0, 0, inlines=(P(3, 1), P(3, 2), P(3, 3)))
            # icc1 sweep; rows 0..1023 of att are complete, so the first
            # output-projection half + its ReduceScatter hide under it.
            for g in range(G):
                nc.gpsimd.dma_start(out=wo_sb[g],
                                    in_=wo_in[g * 128:(g + 1) * 128, :])
            attention(3, 0, 1, inlines=(P(0, 6), P(0, 7)))
            for g in range(G):
                if g < G - 1:
                    attention(g, 1, 0, inlines=(P(g + 1, 6), P(g + 1, 7)))
                else:
                    attention(g, 1, 0)
                outproj_m(2 * g)
                attention(g, 1, 1)
                outproj_m(2 * g + 1)
                if g == 1:
                    rs_quarter(0)
                if g == 3:
                    rs_quarter(1)
            for m in range(8, 16):
                outproj_m(m)
            nc.gpsimd.collective_compute(
                "ReduceScatter", OP.add,
                replica_groups=[[0, 1], [2, 3], [4, 5], [6, 7]],
                ins=[part_dram[1024:2048, :]],
                outs=[rs_dram[512:1024, :]])
            for q in range(4):
                r0 = 512 + q * 128
                ob = op_out.tile([128, DIM], f16, tag="ob", name="ob")
                nc.gpsimd.dma_start(out=ob, in_=rs_dram[r0:r0 + 128, :])
                nc.gpsimd.dma_start(out=out_ext[r0:r0 + 128, :], in_=ob)

def kernel(x, context_mask, gamma, null_kv, w_q, w_kv, w_out):
    from concourse.bass_utils import run_bass_kernel_spmd

    x = np.asarray(x, dtype=np.float32)
    context_mask = np.asarray(context_mask)
    gamma = np.asarray(gamma, dtype=np.float32)
    null_kv = np.asarray(null_kv, dtype=np.float32)
    w_q = np.asarray(w_q, dtype=np.float32)
    w_kv = np.asarray(w_kv, dtype=np.float32)
    w_out = np.asarray(w_out, dtype=np.float32)

    if "nc" not in _CACHE:
        _CACHE["nc"] = _build()
    nc = _CACHE["nc"]

    bf = ml_dtypes.bfloat16
    wq_eff = w_q * (gamma[:, None] * SCALE)
    wk_full = w_kv[:, :INNER] * gamma[:, None]
    wv_full = w_kv[:, INNER:] * gamma[:, None]
    nk = null_kv[0, :, 0, :]   # [16, 64]
    nv = null_kv[1, :, 0, :]   # [16, 64]

    in_maps = []
    for core in range(N_CORES):
        b, hg = core // 2, core % 2
        hs = slice(hg * 512, (hg + 1) * 512)
        nk_c = nk[hg * 8:(hg + 1) * 8]          # [8, 64]
        nk65 = np.zeros((G, 128, DH + 1), dtype=np.float32)
        for g in range(G):
            nk65[g, 0:DH, 0] = nk_c[2 * g]
            nk65[g, DH:128, DH] = nk_c[2 * g + 1]
        in_maps.append({
            "x": np.ascontiguousarray(x[b]),
            "wq": np.ascontiguousarray(wq_eff[:, hs]).astype(bf),
            "wk": np.ascontiguousarray(wk_full[:, hs]).astype(bf),
            "wv": np.ascontiguousarray(wv_full[:, hs]).astype(bf),
            "wo": np.ascontiguousarray(w_out[hs, :]).astype(bf),
            "nk": nk65.astype(bf),
            "nv": np.concatenate([nv[hg * 8:(hg + 1) * 8], np.ones((H, 1), np.float32)], axis=1)[None].astype(bf),
            "maskf": np.ascontiguousarray(context_mask[b]).astype(np.float32),
        })

    _CACHE["in_maps"] = in_maps
    res = run_bass_kernel_spmd(nc, in_maps, core_ids=list(range(N_CORES)))

    out = np.empty((B, N, DIM), dtype=np.float32)
    for core in range(N_CORES):
        b, hg = core // 2, core % 2
        r = res.results[core]["out"].astype(np.float32)
        for qr in range(2):
            out[b, qr * 512 + hg * 256:qr * 512 + (hg + 1) * 256, :] = \
                r[qr * 256:(qr + 1) * 256]
        out[b, 1024 + hg * 512:1024 + (hg + 1) * 512, :] = r[512:]
    return out


# revision 86
# speedup vs baseline: 1.0074x; 1.0074x over previous
"""Trainium2 Bass kernel for nn_Attention (dense transformer self-attention block).

Reference computation (fp32):
    xn = LayerNorm(x) * gamma
    q = (xn @ w_q) * SCALE ; k, v = split(xn @ w_kv, 2)
    k/v get a null key/value prepended; sim = q @ k^T; masked softmax
    out = (softmax(sim) @ v) @ w_out

Sharding: 8 cores = 4 batches x 2 head-groups. Core (b, hg) computes
LayerNorm + q/k/v projections + attention for batch b and heads
hg*8..hg*8+8 (attention is fully local to a batch), then projects all 2048
rows through its 512 rows of w_out; a pairwise ReduceScatter sums the two
partial outputs and leaves each core with its half of the rows.

All matmuls run in bf16 with fp32 PSUM accumulation; LayerNorm statistics and
softmax denominators stay fp32.  Softmax is computed without max-subtraction
(sim ~ N(0,1) here, exp cannot overflow) as exp(sim) divided by the row sum,
which is obtained by appending an all-ones column to v.  The context mask and
the null key are folded into the (padded) key/value tensors, which reproduces
the reference masking exactly.
"""

import numpy as np
import ml_dtypes

B = 4
N = 2048
DIM = 1024
H_ALL = 16
DH = 64
INNER = H_ALL * DH
SCALE = DH ** -0.5
EPS = 1e-5

H = 8            # heads per core
G = 4            # head groups of 2 per core (128 partitions each)
R = N            # rows (sequence) per batch
J = 2048         # key slots (the null key is handled as a rank-1 update)
NT_J = J // 128  # 16
IC = 1024        # attention i-chunk (query columns per pass)
N_CORES = 8

_CACHE = {}


def _build(repeat=1):
    from contextlib import ExitStack, nullcontext
    from concourse import bacc
    import concourse.tile as tile
    from concourse import mybir

    f32 = mybir.dt.float32
    bf16 = mybir.dt.bfloat16
    AF = mybir.ActivationFunctionType
    OP = mybir.AluOpType

    nc = bacc.Bacc()

    x_in = nc.declare_dram_parameter("x", [R, DIM], f32, isOutput=False)
    wq_in = nc.declare_dram_parameter("wq", [DIM, 512], bf16, isOutput=False)
    wk_in = nc.declare_dram_parameter("wk", [DIM, 512], bf16, isOutput=False)
    wv_in = nc.declare_dram_parameter("wv", [DIM, 512], bf16, isOutput=False)
    wo_in = nc.declare_dram_parameter("wo", [512, DIM], bf16, isOutput=False)
    nk_in = nc.declare_dram_parameter("nk", [G, 128, DH + 1], bf16, isOutput=False)
    nv_in = nc.declare_dram_parameter("nv", [1, H, DH + 1], bf16, isOutput=False)
    mask_in = nc.declare_dram_parameter("maskf", [R], f32, isOutput=False)
    out_ext = nc.declare_dram_parameter("out", [N // 2, DIM], mybir.dt.float16, isOutput=True)

    xn_dram = nc.dram_tensor("xn_dram", [R, DIM], bf16)
    recip_dram = nc.dram_tensor("recip_dram", [H * 2, IC], mybir.dt.float32)
    part_dram = nc.dram_tensor("part_dram", [R, DIM], mybir.dt.float16)
    rs_dram = nc.dram_tensor("rs_dram", [N // 2, DIM], mybir.dt.float16)

    with tile.TileContext(nc) as tc, ExitStack() as ctx:
        # ---- persistent SBUF tensors --------------------------------------
        persist = ctx.enter_context(tc.tile_pool(name="persist", bufs=1))
        wq_sb = [persist.tile([128, 512], bf16, tag=f"wq{t}", name=f"wq{t}") for t in range(8)]
        wk_sb = [persist.tile([128, 512], bf16, tag=f"wk{t}", name=f"wk{t}") for t in range(8)]
        wv_sb = [persist.tile([128, 512], bf16, tag=f"wv{t}", name=f"wv{t}") for t in range(8)]
        wo_sb = [persist.tile([128, DIM], bf16, tag=f"wo{t}", name=f"wo{t}") for t in range(G)]
        qT_sb = [persist.tile([128, R], bf16, tag=f"qT{g}", name=f"qT{g}") for g in range(G)]
        kT_sb = [persist.tile([128, J], bf16, tag=f"kT{g}", name=f"kT{g}") for g in range(G)]
        v_sb = [persist.tile([128, H, DH + 1], bf16, tag=f"v{t}", name=f"v{t}") for t in range(16)]
        att_sb = [persist.tile([128, R], bf16, tag=f"att{g}", name=f"att{g}") for g in range(G)]
        mask_sb = persist.tile([128, 16], f32, tag="mask", name="mask")
        nk_sb = persist.tile([128, G, DH + 1], bf16, tag="nk", name="nk")
        nv_sb = persist.tile([128, H, DH + 1], bf16, tag="nv", name="nv")


        nc.sync.dma_start(out=nv_sb[0:1], in_=nv_in[:])
        nc.sync.dma_start(out=nv_sb[64:65], in_=nv_in[:])

        # ---- LayerNorm + write xn (bf16) back to DRAM ---------------------
        eps_t = persist.tile([128, 1], f32, tag="eps", name="eps")
        nc.vector.memset(eps_t, EPS)
        xnT = [persist.tile([128, R], bf16, tag=f"xnT{c}", name=f"xnT{c}")
               for c in range(8)]
        for _rep in range(repeat):
            _phases(nc, tc, tile, mybir, locals())

    nc.finalize()
    return nc


def _phases(nc, tc, tile, mybir, env):
    f32 = mybir.dt.float32
    f16 = mybir.dt.float16
    bf16 = mybir.dt.bfloat16
    AF = mybir.ActivationFunctionType
    OP = mybir.AluOpType
    (x_in, xn_dram, part_dram, rs_dram, out_ext) = (
        env["x_in"], env["xn_dram"], env["part_dram"], env["rs_dram"], env["out_ext"])
    (wq_sb, wk_sb, wv_sb, wo_sb, qT_sb, kT_sb, v_sb, att_sb, mask_sb, nk_sb,
     nv_sb, eps_t, xnT) = (
        env["wq_sb"], env["wk_sb"], env["wv_sb"], env["wo_sb"], env["qT_sb"],
        env["kT_sb"], env["v_sb"], env["att_sb"], env["mask_sb"], env["nk_sb"],
        env["nv_sb"], env["eps_t"], env["xnT"])
    recip_dram = env["recip_dram"]
    (wq_in, wk_in, wv_in, wo_in, mask_in, nk_in) = (
        env["wq_in"], env["wk_in"], env["wv_in"], env["wo_in"],
        env["mask_in"], env["nk_in"])

    # ---- LayerNorm (per 128-row tile) + chunked DMA-xbar transposes -------
    # Transposes are split by 512-row chunks so they start as soon as the
    # first four LN tiles have landed in DRAM, and projections start as soon
    # as the first chunks of xnT exist.
    with tc.tile_pool(name="ln", bufs=6) as ln_pool, \
         tc.tile_pool(name="ln_small", bufs=6) as ln_small:
        for t in range(R // 128):
            x_t = ln_pool.tile([128, DIM], f32, tag="x", bufs=12)
            eng = nc.sync if t % 2 == 0 else nc.scalar
            eng.dma_start(out=x_t, in_=x_in[t * 128:(t + 1) * 128, :])
            stats = ln_small.tile([128, 2, 6], f32, tag="stats")
            for s in range(2):
                nc.vector.bn_stats(out=stats[:, s, :],
                                   in_=x_t[:, s * 512:(s + 1) * 512])
            mv = ln_small.tile([128, 2], f32, tag="mv")
            nc.vector.bn_aggr(out=mv, in_=stats)
            rstd = ln_small.tile([128, 1], f32, tag="rstd")
            nc.scalar.activation(out=rstd, in_=mv[:, 1:2], func=AF.Sqrt,
                                 bias=eps_t, scale=1.0)
            nc.vector.reciprocal(out=rstd, in_=rstd)
            nbias = ln_small.tile([128, 1], f32, tag="nbias")
            nc.vector.tensor_scalar(out=nbias, in0=rstd,
                                    scalar1=mv[:, 0:1], scalar2=-1.0,
                                    op0=OP.mult, op1=OP.mult)
            xn_t = ln_pool.tile([128, DIM], bf16, tag="xn")
            nc.scalar.activation(out=xn_t, in_=x_t, func=AF.Identity,
                                 bias=nbias, scale=rstd)
            if t == 4:
                for wt in range(8):
                    nc.gpsimd.dma_start(out=wk_sb[wt],
                                        in_=wk_in[wt * 128:(wt + 1) * 128, :])
                    nc.gpsimd.dma_start(out=wq_sb[wt],
                                        in_=wq_in[wt * 128:(wt + 1) * 128, :])
                nc.sync.dma_start(out=mask_sb,
                                  in_=mask_in.rearrange("(t p) -> p t", p=128))
                nc.sync.dma_start(out=nk_sb,
                                  in_=nk_in.rearrange("g p c -> p g c"))
                for wt in range(8):
                    nc.gpsimd.dma_start(out=wv_sb[wt],
                                        in_=wv_in[wt * 128:(wt + 1) * 128, :])
            nc.sync.dma_start(out=xn_dram[t * 128:(t + 1) * 128, :], in_=xn_t)
            if t % 2 == 1:
                rc = t // 2
                for c in range(8):
                    nc.sync.dma_start_transpose(
                        out=xnT[c][:, rc * 256:(rc + 1) * 256],
                        in_=xn_dram[rc * 256:(rc + 1) * 256,
                                    c * 128:(c + 1) * 128])

    # ---- projections + attention, interleaved per head-group -------------
    # v first (attention needs all of v), then per group g: k[g], q[g],
    # attention for g.  The scheduler overlaps the next group's projections
    # with the current group's (ACT-bound) attention inner loop.
    with tc.tile_pool(name="ps_proj", bufs=2, space="PSUM") as ps_proj, \
         tc.tile_pool(name="ps_sim", bufs=2, space="PSUM") as ps_sim, \
         tc.tile_pool(name="ps_av", bufs=1, space="PSUM") as ps_av, \
         tc.tile_pool(name="exp_pool", bufs=8) as exp_pool, \
         tc.tile_pool(name="att_small", bufs=4) as att_small:

        def project_v(jt):
            # v: [key rows on partitions, 8 heads x 64 dims free], mask folded
            ps = ps_proj.tile([128, 512], f32, tag="ps", name="ps")
            for kt in range(8):
                nc.tensor.matmul(
                    ps,
                    lhsT=xnT[kt][:, jt * 128:(jt + 1) * 128],
                    rhs=wv_sb[kt],
                    start=(kt == 0), stop=(kt == 7))
            nc.vector.tensor_scalar_mul(
                out=v_sb[jt][:, :, 0:DH],
                in0=ps.rearrange("p (l d) -> p l d", l=H),
                scalar1=mask_sb[:, jt:jt + 1])
            nc.vector.memset(v_sb[jt][:, :, DH:DH + 1], 0.0)
            nc.vector.tensor_scalar_add(
                out=v_sb[jt][:, :, DH:DH + 1],
                in0=v_sb[jt][:, :, DH:DH + 1],
                scalar1=mask_sb[:, jt:jt + 1])

        def project_kq_piece(g, piece):
            # k[g] / q[g] one (dst, rc) chunk: [2-head dims on parts, rows]
            dst_sb, w_sb = ((kT_sb, wk_sb), (qT_sb, wq_sb))[piece // 4]
            rc = piece % 4
            ps = ps_proj.tile([128, 512], f32, tag="ps", name="ps")
            for kt in range(8):
                nc.tensor.matmul(
                    ps,
                    lhsT=w_sb[kt][:, g * 128:(g + 1) * 128],
                    rhs=xnT[kt][:, rc * 512:(rc + 1) * 512],
                    start=(kt == 0), stop=(kt == 7))
            nc.vector.tensor_copy(
                out=dst_sb[g][:, rc * 512:(rc + 1) * 512], in_=ps)


        def P(g, piece):
            return lambda: project_kq_piece(g, piece)

        def V(jt):
            return lambda: project_v(jt)



        null_expn = {}

        def attention(g, icc, w, inlines=()):
            l = 2 * g + w  # local head index
            i0 = icc * IC
            av = ps_av.tile([DH + 1, IC], f32, tag="av", name="av")
            for jt in range(NT_J):
                if 1 <= jt <= len(inlines):
                    inlines[jt - 1]()
                sim = ps_sim.tile([128, IC], f32, tag="sim", name="sim")
                _sv = tc.cur_priority
                tc.cur_priority = max(0, _sv - 250)
                for nn in range(IC // 512):
                    nc.tensor.matmul(
                        sim[:, nn * 512:(nn + 1) * 512],
                        lhsT=kT_sb[g][w * 64:(w + 1) * 64,
                                      jt * 128:(jt + 1) * 128],
                        rhs=qT_sb[g][w * 64:(w + 1) * 64,
                                     i0 + nn * 512:i0 + (nn + 1) * 512],
                        start=True, stop=True,
                        skip_group_check=True)
                tc.cur_priority = _sv
                exp_t = exp_pool.tile([128, IC], bf16, tag="exp", name="exp")
                nc.scalar.activation(out=exp_t, in_=sim, func=AF.Exp)
                for nn in range(IC // 512):
                    nc.tensor.matmul(
                        av[:, nn * 512:(nn + 1) * 512],
                        lhsT=v_sb[jt][:, l, :],
                        rhs=exp_t[:, nn * 512:(nn + 1) * 512],
                        start=(jt == 0), stop=False,
                        skip_group_check=True)
            if w == 0:
                # null-key sims for BOTH heads of the group in one matmul:
                # head w's sim lands on psum partition w*64 (nk65 layout).
                nsim = ps_sim.tile([128, IC], f32, tag="sim", name="nsim")
                _sv2 = tc.cur_priority
                tc.cur_priority = max(0, _sv2 - 250)
                for nn in range(IC // 512):
                    nc.tensor.matmul(
                        nsim[0:DH + 1, nn * 512:(nn + 1) * 512],
                        lhsT=nk_sb[:, g, :],
                        rhs=qT_sb[g][:, i0 + nn * 512:i0 + (nn + 1) * 512],
                        start=True, stop=True, skip_group_check=True)
                tc.cur_priority = _sv2
                expn = exp_pool.tile([128, IC], bf16, tag="exp", name="expn")
                nc.scalar.activation(out=expn[0:DH + 1, :],
                                     in_=nsim[0:DH + 1, :], func=AF.Exp)
                null_expn[(g, icc)] = expn
            expn = null_expn[(g, icc)]
            for nn in range(IC // 512):
                nc.tensor.matmul(
                    av[:, nn * 512:(nn + 1) * 512],
                    lhsT=nv_sb[w * 64:w * 64 + 1, l, :],
                    rhs=expn[w * 64:w * 64 + 1, nn * 512:(nn + 1) * 512],
                    start=False, stop=True, skip_group_check=True)
            # evict unnormalized (frees the psum accumulator fast), save the
            # denominator; the reciprocal/broadcast/multiply runs off the
            # critical path on DVE + DMA.
            nc.vector.tensor_copy(
                out=att_sb[g][w * 64:(w + 1) * 64, i0:i0 + IC],
                in_=av[0:DH, :])
            recip = att_small.tile([1, IC], f32, tag="recip", name="recip")
            nc.vector.reciprocal(out=recip, in_=av[DH:DH + 1, :])
            slot = recip_dram[l * 2 + icc:l * 2 + icc + 1, :]
            nc.sync.dma_start(out=slot, in_=recip)
            bcast = att_small.tile([128, IC], f32, tag="bcast", name="bcast")
            bslice = bcast[w * 64:(w + 1) * 64, :]
            nc.sync.dma_start(out=bslice, in_=slot.to_broadcast([64, IC]))
            nc.vector.tensor_mul(
                out=att_sb[g][w * 64:(w + 1) * 64, i0:i0 + IC],
                in0=att_sb[g][w * 64:(w + 1) * 64, i0:i0 + IC], in1=bslice)

        def outproj_m(m):
            for oc in range(2):
                ps = ps_proj.tile([128, 512], f32, tag="ps", name="pso")
                for g in range(G):
                    nc.tensor.matmul(
                        ps,
                        lhsT=att_sb[g][:, m * 128:(m + 1) * 128],
                        rhs=wo_sb[g][:, oc * 512:(oc + 1) * 512],
                        start=(g == 0), stop=(g == G - 1))
                o_t = op_out.tile([128, 512], f16, tag="ot", name="ot")
                nc.vector.tensor_copy(out=o_t, in_=ps)
                nc.sync.dma_start(
                    out=part_dram[m * 128:(m + 1) * 128,
                                  oc * 512:(oc + 1) * 512],
                    in_=o_t)

        def rs_quarter(qr):
            nc.gpsimd.collective_compute(
                "ReduceScatter", OP.add,
                replica_groups=[[0, 1], [2, 3], [4, 5], [6, 7]],
                ins=[part_dram[qr * 512:(qr + 1) * 512, :]],
                outs=[rs_dram[qr * 256:(qr + 1) * 256, :]])
            for q in range(2):
                r0 = qr * 256 + q * 128
                ob = op_out.tile([128, DIM], f16, tag="ob", name="ob")
                nc.gpsimd.dma_start(out=ob, in_=rs_dram[r0:r0 + 128, :])
                nc.gpsimd.dma_start(out=out_ext[r0:r0 + 128, :], in_=ob)

        with tc.tile_pool(name="op_out", bufs=6) as op_out:
            # icc0 sweep with just-in-time k/q projections; v split around the
            # first attention block so the exp stream starts early and never
            # starves while v finishes.
            for pc in (0, 4, 5):   # k0 rc0, q0 rc0, q0 rc1
                project_kq_piece(0, pc)
            for jt in range(8):
                project_v(jt)
            attention(0, 0, 0, inlines=(
                P(0, 1), P(0, 2), P(0, 3),
                V(8), V(9), V(10), V(11), V(12), V(13), V(14), V(15)))
            attention(0, 0, 1, inlines=(P(1, 0), P(1, 4), P(1, 5)))
            attention(1, 0, 0, inlines=(P(1, 1), P(1, 2), P(1, 3)))
            attention(1, 0, 1, inlines=(P(2, 0), P(2, 4), P(2, 5)))
            attention(2, 0, 0, inlines=(P(2, 1), P(2, 2), P(2, 3)))
            attention(2, 0, 1, inlines=(P(3, 0), P(3, 4), P(3, 5)))
            attention(3, 0, 0, inlines=(P(3, 1), P(3, 2), P(3, 3)))
            # icc1 sweep; rows 0..1023 of att are complete, so the first
            # output-projection half + its ReduceScatter hide under it.
            for g in range(G):
                nc.gpsimd.dma_start(out=wo_sb[g],
                                    in_=wo_in[g * 128:(g + 1) * 128, :])
            attention(3, 0, 1, inlines=(P(0, 6), P(0, 7)))
            for g in range(G):
                if g < G - 1:
                    attention(g, 1, 0, inlines=(P(g + 1, 6), P(g + 1, 7)))
                else:
                    attention(g, 1, 0)
                outproj_m(2 * g)
                attention(g, 1, 1)
                outproj_m(2 * g + 1)
                if g == 1:
                    rs_quarter(0)
                if g == 3:
                    rs_quarter(1)
            for m in range(8, 16):
                outproj_m(m)
            nc.gpsimd.collective_compute(
                "ReduceScatter", OP.add,
                replica_groups=[[0, 1], [2, 3], [4, 5], [6, 7]],
                ins=[part_dram[1024:2048, :]],
                outs=[rs_dram[512:1024, :]])
            for q in range(4):
                r0 = 512 + q * 128
                eng = (nc.gpsimd, nc.sync, nc.scalar, nc.gpsimd)[q]
                ob = op_out.tile([128, DIM], f16, tag="ob", name="ob")
                eng.dma_start(out=ob, in_=rs_dram[r0:r0 + 128, :])
                eng.dma_start(out=out_ext[r0:r0 + 128, :], in_=ob)

def kernel(x, context_mask, gamma, null_kv, w_q, w_kv, w_out):
    from concourse.bass_utils import run_bass_kernel_spmd

    x = np.asarray(x, dtype=np.float32)
    context_mask = np.asarray(context_mask)
    gamma = np.asarray(gamma, dtype=np.float32)
    null_kv = np.asarray(null_kv, dtype=np.float32)
    w_q = np.asarray(w_q, dtype=np.float32)
    w_kv = np.asarray(w_kv, dtype=np.float32)
    w_out = np.asarray(w_out, dtype=np.float32)

    if "nc" not in _CACHE:
        _CACHE["nc"] = _build()
    nc = _CACHE["nc"]

    bf = ml_dtypes.bfloat16
    wq_eff = w_q * (gamma[:, None] * SCALE)
    wk_full = w_kv[:, :INNER] * gamma[:, None]
    wv_full = w_kv[:, INNER:] * gamma[:, None]
    nk = null_kv[0, :, 0, :]   # [16, 64]
    nv = null_kv[1, :, 0, :]   # [16, 64]

    in_maps = []
    for core in range(N_CORES):
        b, hg = core // 2, core % 2
        hs = slice(hg * 512, (hg + 1) * 512)
        nk_c = nk[hg * 8:(hg + 1) * 8]          # [8, 64]
        nk65 = np.zeros((G, 128, DH + 1), dtype=np.float32)
        for g in range(G):
            nk65[g, 0:DH, 0] = nk_c[2 * g]
            nk65[g, DH:128, DH] = nk_c[2 * g + 1]
        in_maps.append({
            "x": np.ascontiguousarray(x[b]),
            "wq": np.ascontiguousarray(wq_eff[:, hs]).astype(bf),
            "wk": np.ascontiguousarray(wk_full[:, hs]).astype(bf),
            "wv": np.ascontiguousarray(wv_full[:, hs]).astype(bf),
            "wo": np.ascontiguousarray(w_out[hs, :]).astype(bf),
            "nk": nk65.astype(bf),
            "nv": np.concatenate([nv[hg * 8:(hg + 1) * 8], np.ones((H, 1), np.float32)], axis=1)[None].astype(bf),
            "maskf": np.ascontiguousarray(context_mask[b]).astype(np.float32),
        })

    _CACHE["in_maps"] = in_maps
    res = run_bass_kernel_spmd(nc, in_maps, core_ids=list(range(N_CORES)))

    out = np.empty((B, N, DIM), dtype=np.float32)
    for core in range(N_CORES):
        b, hg = core // 2, core % 2
        r = res.results[core]["out"].astype(np.float32)
        for qr in range(2):
            out[b, qr * 512 + hg * 256:qr * 512 + (hg + 1) * 256, :] = \
                r[qr * 256:(qr + 1) * 256]
        out[b, 1024 + hg * 512:1024 + (hg + 1) * 512, :] = r[512:]
    return out
